# revision 17
# baseline (speedup 1.0000x reference)
"""AF2GNN1 distributed Trainium2 kernel.

Sharding: pixels (65536) row-sharded 8192/core for the two big Q matmuls;
nodes (2048) sharded 256/core for the GNN middle (ReduceScatter slices,
AllGathers rebuild full-N stationaries). GNN runs in transposed layout
(features on partitions, local nodes on the free axis). Phase A writes a
bf16 copy of Q so phase C can load Q.T tiles via bf16 DMA-transpose.
"""
import sys

sys.path.insert(0, "/opt/trn_rl_repo")

import numpy as np
import ml_dtypes
from contextlib import ExitStack

import concourse.bass as bass
import concourse.mybir as mybir
import concourse.tile as tile
from concourse import bacc
from concourse.bass_utils import run_bass_kernel_spmd
from concourse.masks import make_identity

F32 = mybir.dt.float32
BF16 = mybir.dt.bfloat16
AF = mybir.ActivationFunctionType
ALU = mybir.AluOpType

NCORES = 8
HW = 65536
C = 200          # channels (B in reference)
N = 2048         # superpixels / nodes
NHID = 128
OUT = 64
HEADS = 4
P = HW // NCORES          # 8192 pixels per core
PT = P // 128             # 64 pixel tiles
NS = N // NCORES          # 256 nodes per core
JT = N // 128             # 16 node tiles
BN = 1.0 / np.sqrt(1.0 + 1e-5)
CP = 225          # x channels (200) + zero pad (24) + ones col at 224
C1W = CP - 128    # chunk-1 width (97; colsum at chunk-1 partition 96)

_CACHE = {}


def _build():
    nc = bacc.Bacc("TRN2", target_bir_lowering=False, debug=False,
                   num_devices=NCORES)

    def din(name, shape, dt):
        return nc.dram_tensor(name, list(shape), dt, kind="ExternalInput")

    xq_d = din("xq", [P, CP], BF16)        # x shard + pad + ones column
    q_d = din("q", [P, N], BF16)               # Q pixel shard (bf16)
    qt_d = din("qt", [N, P], BF16)             # Q pixel shard, pre-transposed
    adjT_d = din("adjT", [N, NS], BF16)        # adj[shard].T
    w1a_d = din("w1a", [C, NHID], BF16)        # BN^3 * g1a_W
    w1b_d = din("w1b", [NHID, C], BF16)        # BN * g1b_W
    w2a_d = din("w2a", [C, NHID], BF16)        # BN * g2a_W
    w2b_d = din("w2b", [NHID, OUT], BF16)      # BN * g2b_W
    gatw_d = din("gatw", [C, HEADS * NHID], BF16)   # BN * gat_W, heads on cols
    gata1_d = din("gata1", [NHID, HEADS * 128], BF16)  # a1_k replicated cols
    gata2_d = din("gata2", [NHID, HEADS], BF16)        # a2_k columns
    outw_d = din("outw", [HEADS * NHID, OUT], BF16)
    outa1_d = din("outa1", [OUT, 128], BF16)   # out_a[:64] replicated
    outa2_d = din("outa2", [OUT, 1], BF16)     # out_a[64:]
    b1a_d = din("b1a", [NHID, 1], F32)
    b1b_d = din("b1b", [C, 1], F32)            # BN * g1b_b
    b2a_d = din("b2a", [NHID, 1], F32)
    b2b_d = din("b2b", [OUT, 1], F32)          # BN * g2b_b
    fuse_d = din("fuse", [OUT, 6], F32)  # wfa, wfb, cbv, cw0, cw1, cwsum
    out_d = nc.dram_tensor("out", [OUT, P], F32, kind="ExternalOutput")

    RG = [list(range(NCORES))]

    with tile.TileContext(nc) as tc, ExitStack() as ctx:
        _sc = [None]

        def mark(name):
            if _sc[0] is not None:
                _sc[0].__exit__(None, None, None)
                _sc[0] = None
            if name:
                _sc[0] = tc.spectator_scope(name)
                _sc[0].__enter__()

        dram = ctx.enter_context(tc.tile_pool(name="dram", bufs=1, space="DRAM"))

        # ---- constants / weights ----
        cons = ctx.enter_context(tc.tile_pool(name="cons", bufs=1))
        idf = cons.tile([128, 128], F32)
        make_identity(nc, idf[:, :])
        idb = cons.tile([128, 128], BF16)
        make_identity(nc, idb[:, :])
        ones_row = cons.tile([1, 128], BF16)
        nc.vector.memset(ones_row[:, :], 1.0)
        ones_row_f = cons.tile([1, 128], F32)
        nc.vector.memset(ones_row_f[:, :], 1.0)
        ones_col = cons.tile([128, 1], BF16)
        nc.vector.memset(ones_col[:, :], 1.0)
        negbn = cons.tile([64, 1], F32)
        nc.vector.memset(negbn[:, :], -BN)

        _ld = [0]
        def load(dst_shape, src, dt=BF16, eng=None):
            _ld[0] += 1
            t = cons.tile(dst_shape, dt, name=f"w{_ld[0]}")
            (eng or nc.gpsimd).dma_start(t[:], src)
            return t

        C1 = C - 128
        w1a = (load([128, NHID], w1a_d[0:128, :]),
               load([C1, NHID], w1a_d[128:C, :]))
        w1b = load([NHID, C], w1b_d[:, :])
        w2a = (load([128, NHID], w2a_d[0:128, :]),
               load([C1, NHID], w2a_d[128:C, :]))
        w2b = load([NHID, OUT], w2b_d[:, :])
        gatw = (load([128, HEADS * NHID], gatw_d[0:128, :]),
                load([C1, HEADS * NHID], gatw_d[128:C, :]))
        gata1 = load([NHID, HEADS * 128], gata1_d[:, :])
        gata2 = load([NHID, HEADS], gata2_d[:, :])
        outw = [load([128, OUT], outw_d[c4 * 128:(c4 + 1) * 128, :])
                for c4 in range(HEADS)]
        outa1 = load([OUT, 128], outa1_d[:, :])
        outa2 = load([OUT, 1], outa2_d[:, :])
        b1a = load([NHID, 1], b1a_d[:, :], F32)
        b1b = (load([128, 1], b1b_d[0:128, :], F32),
               load([C1, 1], b1b_d[128:C, :], F32))
        b2a = load([NHID, 1], b2a_d[:, :], F32)
        b2b = load([OUT, 1], b2b_d[:, :], F32)
        fuse = load([OUT, 6], fuse_d[:, :], F32)
        adjT = cons.tile([128, JT, NS], BF16)
        nc.gpsimd.dma_start(adjT[:, :, :],
                            adjT_d[:, :].rearrange("(jt p) i -> p jt i", p=128))

        sbC = ctx.enter_context(tc.tile_pool(name="sbC", bufs=1))
        qtts = [[None] * 4 for _ in range(JT)]
        mark("A")
        # ---- phase A: sprawT[c, n] = sum_p xq[p, c] * Q[p, n] in PSUM ----
        rsIn = dram.tile([NCORES * CP, NS], F32)
        with tc.tile_pool(name="sbr", bufs=1) as sbr:
          with tc.tile_pool(name="psA", bufs=1, space="PSUM") as psA, \
               tc.tile_pool(name="sbA", bufs=3) as sbA:
            psA0 = psA.tile([128, N], F32, tag="a0")
            psA1 = psA.tile([128, N], F32, tag="a1")
            NI = PT // 2
            for i in range(NI):
                qt = sbA.tile([128, 2, N], BF16, tag="qt")
                nc.scalar.dma_start(
                    qt[:, :, :],
                    q_d[i * 256:(i + 1) * 256, :].rearrange(
                        "(t p) n -> p t n", p=128))
                xt = sbA.tile([128, 2, CP], BF16, tag="xt")
                nc.gpsimd.dma_start(
                    xt[:, :, :],
                    xq_d[i * 256:(i + 1) * 256, :].rearrange(
                        "(t p) n -> p t n", p=128))
                for t2 in range(2):
                    st, sp = (i == 0 and t2 == 0), (i == NI - 1 and t2 == 1)
                    for nb in range(4):
                        nsl = slice(nb * 512, (nb + 1) * 512)
                        nc.tensor.matmul(psA0[0:64, nsl], xt[:, t2, 0:64],
                                         qt[:, t2, nsl], start=st, stop=sp)
                    for nb in range(4):
                        nsl = slice(nb * 512, (nb + 1) * 512)
                        nc.tensor.matmul(psA0[64:128, nsl], xt[:, t2, 64:128],
                                         qt[:, t2, nsl], start=st, stop=sp)
                    for nb in range(4):
                        nsl = slice(nb * 512, (nb + 1) * 512)
                        nc.tensor.matmul(psA1[0:C1W, nsl],
                                         xt[:, t2, 128:CP], qt[:, t2, nsl],
                                         start=st, stop=sp)

            # copy PSUM -> SBUF, transpose to natural [n, c], write RS input
            spr0 = sbr.tile([128, N], F32)
            nc.vector.tensor_copy(spr0[:, :], psA0[:, :])
            spr1 = sbr.tile([C1W, N], F32)
            nc.vector.tensor_copy(spr1[:, :], psA1[0:C1W, :])

            # prefetch Q.T tiles for phase C from the host-pretransposed
            # copy; ring of QTB buffers drains during the GNN + phase C
            QP = P // 4
            for qr in range(4):
                for jt in range(JT):
                    qq = sbC.tile([128, QP], BF16, tag="qtt", bufs=20,
                                  name=f"qtt{jt}_{qr}")
                    nc.sync.dma_start(
                        qq[:, :],
                        qt_d[jt * 128:(jt + 1) * 128,
                             qr * QP:(qr + 1) * QP])
                    qtts[jt][qr] = qq

          mark("a2a")
          # all-to-all exchanges column-blocks of sprawT; each core then
          # sums the 8 partials for its node shard (stays in T layout)
          nc.gpsimd.dma_start(
              rsIn[:, :].rearrange("(s c) i -> c s i", c=CP)[0:128],
              spr0[:, :])
          nc.gpsimd.dma_start(
              rsIn[:, :].rearrange("(s c) i -> c s i", c=CP)[128:CP],
              spr1[:, :])

        a2aOut = dram.tile([NCORES * CP, NS], F32)
        nc.gpsimd.collective_compute(
            "AllToAll", ALU.bypass, replica_groups=RG,
            ins=[rsIn.opt()], outs=[a2aOut.opt()])

        # ---- local sp prep: sum partials, normalize, already T layout ----
        gnn = ctx.enter_context(tc.tile_pool(name="gnn", bufs=1))
        spT_f = [gnn.tile([128, NS], F32, name="spTf0"),
                 gnn.tile([C - 128, NS], F32, name="spTf1")]
        spT_b = [gnn.tile([128, NS], BF16, name="spTb0"),
                 gnn.tile([C - 128, NS], BF16, name="spTb1")]

        with tc.tile_pool(name="psP", bufs=2, space="PSUM") as psP, \
             tc.tile_pool(name="sbP", bufs=1) as sbP:
            vout = a2aOut[:, :].rearrange("(s c) i -> c s i", c=CP)
            sp8_0 = sbP.tile([128, NCORES, NS], F32, name="sp80")
            nc.gpsimd.dma_start(sp8_0[:, :, :], vout[0:128])
            sp8_1 = sbP.tile([C1W, NCORES, NS], F32, name="sp81")
            nc.gpsimd.dma_start(sp8_1[:, :, :], vout[128:CP])
            sum0 = sbP.tile([128, NS], F32, name="sum0")
            nc.vector.tensor_reduce(
                sum0[:, :], sp8_0[:, :, :].rearrange("p s i -> p i s"),
                axis=mybir.AxisListType.X, op=ALU.add)
            sum1 = sbP.tile([C1W, NS], F32, name="sum1")
            nc.vector.tensor_reduce(
                sum1[:, :], sp8_1[:, :, :].rearrange("p s i -> p i s"),
                axis=mybir.AxisListType.X, op=ALU.add)
            rec_row = sbP.tile([1, NS], F32, name="recrow")
            nc.vector.reciprocal(rec_row[:, :], sum1[96:97, :])
            recb_ps = psP.tile([128, NS], F32, name="recbps")
            nc.tensor.matmul(recb_ps[:, :], ones_row_f[:, :], rec_row[:, :],
                             start=True, stop=True)
            recb = sbP.tile([128, NS], F32, name="recb")
            nc.vector.tensor_copy(recb[:, :], recb_ps[:, :])
            nc.vector.tensor_mul(spT_f[0][:, :], sum0[:, :], recb[:, :])
            nc.vector.tensor_mul(spT_f[1][:, :], sum1[0:C - 128, :],
                                 recb[0:C - 128, :])
            nc.vector.tensor_copy(spT_b[0][:, :], spT_f[0][:, :])
            nc.vector.tensor_copy(spT_b[1][:, :], spT_f[1][:, :])

        ysb = gnn.tile([128, JT, OUT], BF16)

        mark("gat")
        # ================= GNN =================
        gnn_blk = ExitStack()
        psG = gnn_blk.enter_context(tc.tile_pool(name="psG", bufs=1, space="PSUM"))
        PSB = {"tp": 1, "acc128": 3, "accS": 2, "bc": 2}
        _pn = [0]
        def pstile(shape, dt, tag):
            _pn[0] += 1
            return psG.tile(shape, dt, tag=tag, bufs=PSB[tag],
                            name=f"ps_{tag}_{_pn[0]}")
        sbG = gnn_blk.enter_context(tc.tile_pool(name="sbG", bufs=1))

        cks = [(0, 128), (128, C - 128)]  # feature chunks of C=200

        def mm_c_chunks(out_ps, w_pair, w_csl, rhs_pair, st=True, sp=True):
            # out += w.T @ rhs over the two C-chunks
            nc.tensor.matmul(out_ps, w_pair[0][:, w_csl], rhs_pair[0][:, :],
                             start=st, stop=False)
            nc.tensor.matmul(out_ps, w_pair[1][:, w_csl], rhs_pair[1][:, :],
                             start=False, stop=sp)

        def transpose_to(dst, src_ap, pw, identity):
            # src [pw, 128] -> psum [128, pw] -> copy into dst (cast ok)
            pt = pstile([128, 128], src_ap.dtype, "tp")
            nc.tensor.transpose(pt[0:128, 0:pw], src_ap, identity[0:pw, 0:pw])
            nc.vector.tensor_copy(dst, pt[0:128, 0:pw])

        # ---- GAT heads: local Wh, f1, f2; batch AllGather ----
        AGW = HEADS * NHID + HEADS + NHID  # heads | f2 | t1
        agIn = dram.tile([NS, AGW], BF16)
        agIn_sb = gnn.tile([128, 2, AGW], BF16)
        whT = []
        for k in range(HEADS):
            whp = pstile([128, NS], F32, "acc128")
            mm_c_chunks(whp[:, :], gatw, slice(k * NHID, (k + 1) * NHID), spT_b)
            wb = gnn.tile([128, NS], BF16, name=f"whT{k}")
            nc.vector.tensor_copy(wb[:, :], whp[:, :])
            whT.append(wb)
            for it in range(2):
                isl = slice(it * 128, (it + 1) * 128)
                # f2 column chunk [128, 1]
                f2p = pstile([128, 1], F32, "accS")
                nc.tensor.matmul(f2p[:, :], wb[:, isl],
                                 gata2[:, k:k + 1], start=True, stop=True)
                nc.vector.tensor_copy(
                    agIn_sb[:, it, HEADS * NHID + k:HEADS * NHID + k + 1],
                    f2p[:, :])
                # Wh natural chunk [128, 128]
                transpose_to(agIn_sb[:, it, k * NHID:(k + 1) * NHID],
                             wb[:, isl], 128, idb)
        t1p = pstile([128, NS], F32, "acc128")
        mm_c_chunks(t1p[:, :], w1a, slice(0, NHID), spT_b)
        t1b = sbG.tile([128, NS], BF16, tag="tTb")
        nc.vector.tensor_copy(t1b[:, :], t1p[:, :])
        for it in range(2):
            transpose_to(agIn_sb[:, it, 516:516 + NHID],
                         t1b[:, it * 128:(it + 1) * 128], 128, idb)
        nc.gpsimd.dma_start(
            agIn[:, :].rearrange("(it p) c -> p it c", p=128), agIn_sb[:, :, :])
        agOut = dram.tile([N, AGW], BF16, addr_space="Shared")
        nc.gpsimd.collective_compute(
            "AllGather", ALU.bypass, replica_groups=RG,
            ins=[agIn.opt()], outs=[agOut.opt()])
        agO = gnn.tile([128, JT, AGW], BF16)
        nc.gpsimd.dma_start(
            agO[:, :, :], agOut[:, :].rearrange("(jt p) c -> p jt c", p=128))
        f2f = gnn.tile([128, JT, HEADS], F32)
        nc.vector.tensor_copy(f2f[:, :, :],
                              agO[:, :, HEADS * NHID:HEADS * NHID + HEADS])
        f2s = gnn.tile([128, JT, HEADS], F32)
        nc.vector.tensor_scalar_mul(f2s[:, :, :], f2f[:, :, :], 0.2)

        # ---- GCN block 1 (t1 = W1a.T @ spT; AG; adj-matmul; ...) ----
        def ag_roundtrip(nat_sb, width, name):
            gin = dram.tile([NS, width], BF16, name=f"gin_{name}")
            nc.gpsimd.dma_start(
                gin[:, :].rearrange("(it p) c -> p it c", p=128), nat_sb)
            gout = dram.tile([N, width], BF16, name=f"gout_{name}", addr_space="Shared")
            nc.gpsimd.collective_compute(
                "AllGather", ALU.bypass, replica_groups=RG,
                ins=[gin.opt()], outs=[gout.opt()])
            gsb = gnn.tile([128, JT, width], BF16, name=f"gsb_{name}")
            nc.gpsimd.dma_start(
                gsb[:, :, :],
                gout[:, :].rearrange("(jt p) c -> p jt c", p=128))
            return gsb

        def adj_mm(out_ps, full_sb, width):
            for jt in range(JT):
                nc.tensor.matmul(out_ps, full_sb[:, jt, 0:width],
                                 adjT[:, jt, :],
                                 start=(jt == 0), stop=(jt == JT - 1))

        hcat = []

        def emit_head(k):
            f1b_k = pstile([128, NS], F32, "bc")
            nc.tensor.matmul(f1b_k[:, :], gata1[:, k * 128:(k + 1) * 128],
                             whT[k][:, :], start=True, stop=True)
            unp = pstile([128, NS], F32, "acc128")
            zp = pstile([1, NS], F32, "accS")
            for jb in range(2):
                jts = range(jb * 8, jb * 8 + 8)
                zs, ls, ems = {}, {}, {}
                for jt in jts:
                    zs[jt] = sbG.tile([128, NS], F32, tag="zsb", bufs=9,
                                      name=f"z{k}_{jt}")
                    nc.scalar.activation(zs[jt][:, :], f1b_k[:, :],
                                         AF.Identity,
                                         bias=f2f[:, jt, k:k + 1], scale=1.0)
                for jt in jts:
                    ls[jt] = sbG.tile([128, NS], F32, tag="lsb", bufs=9,
                                      name=f"l{k}_{jt}")
                    nc.vector.scalar_tensor_tensor(
                        ls[jt][:, :], zs[jt][:, :], 0.2, zs[jt][:, :],
                        op0=ALU.mult, op1=ALU.max)
                for jt in jts:
                    zs[jt] = sbG.tile([128, NS], BF16, tag="esb", bufs=9,
                                      name=f"e{k}_{jt}")
                    nc.scalar.activation(zs[jt][:, :], ls[jt][:, :], AF.Exp)
                for jt in jts:
                    ems[jt] = sbG.tile([128, NS], BF16, tag="em", bufs=9,
                                       name=f"m{k}_{jt}")
                    nc.vector.tensor_mul(ems[jt][:, :], zs[jt][:, :],
                                         adjT[:, jt, :])
                for jt in jts:
                    nc.tensor.matmul(unp[:, :],
                                     agO[:, jt, k * NHID:(k + 1) * NHID],
                                     ems[jt][:, :], start=(jt == 0),
                                     stop=(jt == JT - 1))
                    nc.tensor.matmul(zp[:, :], ones_col[:, :], ems[jt][:, :],
                                     start=(jt == 0), stop=(jt == JT - 1))
            ziv = sbG.tile([1, NS], F32, tag="ziv")
            nc.vector.reciprocal(ziv[:, :], zp[:, :])
            zbc = pstile([128, NS], F32, "tp")
            nc.tensor.matmul(zbc[:, :], ones_row_f[:, :], ziv[:, :],
                             start=True, stop=True)
            zbs = sbG.tile([128, NS], F32, tag="zbs")
            nc.vector.tensor_copy(zbs[:, :], zbc[:, :])
            ho = sbG.tile([128, NS], F32, tag="ho")
            nc.vector.tensor_mul(ho[:, :], unp[:, :], zbs[:, :])
            # elu(ho) + 1 = max(ho,0) + exp(min(ho,0))
            mn0 = sbG.tile([128, NS], F32, tag="mn0")
            nc.vector.tensor_scalar_min(mn0[:, :], ho[:, :], 0.0)
            ex = sbG.tile([128, NS], F32, tag="ex")
            nc.scalar.activation(ex[:, :], mn0[:, :], AF.Exp)
            ep = sbG.tile([128, NS], F32, tag="ep")
            nc.vector.scalar_tensor_tensor(ep[:, :], ho[:, :], 0.0, ex[:, :],
                                           op0=ALU.max, op1=ALU.add)
            hc = gnn.tile([128, NS], BF16, name=f"hc{k}")
            nc.vector.tensor_scalar_add(hc[:, :], ep[:, :], -1.0)
            hcat.append(hc)


        mark("gcn")
        u1p = pstile([128, NS], F32, "acc128")
        adj_mm(u1p[:, :], agO[:, :, 516:516 + NHID], NHID)
        z2b = sbG.tile([128, NS], BF16, tag="zTb")
        nc.scalar.activation(z2b[:, :], u1p[:, :], AF.Lrelu,
                             bias=b1a[:, :], scale=1.0, alpha=0.01)

        t2p0 = pstile([128, NS], F32, "acc128")
        nc.tensor.matmul(t2p0[:, :], w1b[:, 0:128], z2b[:, :],
                         start=True, stop=True)
        t2p1 = pstile([128, NS], F32, "acc128")
        nc.tensor.matmul(t2p1[0:C - 128, :], w1b[:, 128:C], z2b[:, :],
                         start=True, stop=True)
        t2b0 = sbG.tile([128, NS], BF16, tag="tTb")
        nc.vector.tensor_copy(t2b0[:, :], t2p0[:, :])
        t2b1 = sbG.tile([C - 128, NS], BF16, tag="tTb2")
        nc.vector.tensor_copy(t2b1[:, :], t2p1[0:C - 128, :])
        t2n = gnn.tile([128, 2, C], BF16)
        for it in range(2):
            isl = slice(it * 128, (it + 1) * 128)
            transpose_to(t2n[:, it, 0:128], t2b0[:, isl], 128, idb)
            transpose_to(t2n[:, it, 128:C], t2b1[:, isl], C - 128, idb)
        ag2 = ag_roundtrip(t2n[:, :, :], C, "ag2")
        emit_head(0)
        r1p0 = pstile([128, NS], F32, "acc128")
        r1p1 = pstile([128, NS], F32, "acc128")
        for jt in range(JT):
            nc.tensor.matmul(r1p0[:, :], ag2[:, jt, 0:128], adjT[:, jt, :],
                             start=(jt == 0), stop=(jt == JT - 1))
            nc.tensor.matmul(r1p1[0:C - 128, :], ag2[:, jt, 128:C],
                             adjT[:, jt, :],
                             start=(jt == 0), stop=(jt == JT - 1))
        # xg2 = BN^2 * spT + lrelu(BN*r1 + b1b', 0.01), as bf16
        xg2b = [sbG.tile([128, NS], BF16, tag="xg0", name="xg2b0"),
                sbG.tile([C - 128, NS], BF16, tag="xg1", name="xg2b1")]
        y1t = sbG.tile([128, NS], F32, tag="y1t")
        nc.scalar.activation(y1t[:, :], r1p0[:, :], AF.Lrelu,
                             bias=b1b[0][:, :], scale=BN, alpha=0.01)
        nc.vector.scalar_tensor_tensor(xg2b[0][:, :], spT_f[0][:, :],
                                       BN * BN, y1t[:, :],
                                       op0=ALU.mult, op1=ALU.add)
        y1t2 = sbG.tile([C - 128, NS], F32, tag="y1t2")
        nc.scalar.activation(y1t2[:, :], r1p1[0:C - 128, :], AF.Lrelu,
                             bias=b1b[1][:, :], scale=BN, alpha=0.01)
        nc.vector.scalar_tensor_tensor(xg2b[1][:, :], spT_f[1][:, :],
                                       BN * BN, y1t2[:, :],
                                       op0=ALU.mult, op1=ALU.add)

        # ---- GCN block 2 ----
        t3p = pstile([128, NS], F32, "acc128")
        mm_c_chunks(t3p[:, :], w2a, slice(0, NHID), xg2b)
        t3b = sbG.tile([128, NS], BF16, tag="tTb")
        nc.vector.tensor_copy(t3b[:, :], t3p[:, :])
        t3n = gnn.tile([128, 2, NHID], BF16)
        for it in range(2):
            transpose_to(t3n[:, it, :], t3b[:, it * 128:(it + 1) * 128],
                         128, idb)
        ag3 = ag_roundtrip(t3n[:, :, :], NHID, "ag3")
        emit_head(1)
        u3p = pstile([128, NS], F32, "acc128")
        adj_mm(u3p[:, :], ag3, NHID)
        z4b = sbG.tile([128, NS], BF16, tag="zTb")
        nc.scalar.activation(z4b[:, :], u3p[:, :], AF.Lrelu,
                             bias=b2a[:, :], scale=1.0, alpha=0.01)
        t4p = pstile([64, NS], F32, "accS")
        nc.tensor.matmul(t4p[:, :], w2b[:, :], z4b[:, :],
                         start=True, stop=True)
        t4b = sbG.tile([64, NS], BF16, tag="t4b")
        nc.vector.tensor_copy(t4b[:, :], t4p[:, :])
        t4n = gnn.tile([128, 2, OUT], BF16)
        for it in range(2):
            transpose_to(t4n[:, it, :], t4b[:, it * 128:(it + 1) * 128],
                         OUT, idb)
        ag4 = ag_roundtrip(t4n[:, :, :], OUT, "ag4")
        emit_head(2)
        gp = pstile([64, NS], F32, "accS")
        adj_mm(gp[:, :], ag4, OUT)
        emit_head(3)
        gcnx = gnn.tile([64, NS], F32)
        nc.scalar.activation(gcnx[:, :], gp[:, :], AF.Lrelu,
                             bias=b2b[:, :], scale=BN, alpha=0.01)


        mark("outatt")
        # ---- output attention ----
        wh2p = pstile([64, NS], F32, "accS")
        for c4 in range(HEADS):
            nc.tensor.matmul(wh2p[:, :], outw[c4][:, :],
                             hcat[c4][:, :], start=(c4 == 0),
                             stop=(c4 == HEADS - 1))
        wh2b = gnn.tile([64, NS], BF16)
        nc.vector.tensor_copy(wh2b[:, :], wh2p[:, :])
        f1ob = pstile([128, NS], F32, "bc")
        nc.tensor.matmul(f1ob[:, :], outa1[:, :], wh2b[:, :],
                         start=True, stop=True)
        agI2 = gnn.tile([128, 2, OUT + 1], BF16)
        for it in range(2):
            isl = slice(it * 128, (it + 1) * 128)
            f2p = pstile([128, 1], F32, "accS")
            nc.tensor.matmul(f2p[:, :], wh2b[:, isl], outa2[:, :],
                             start=True, stop=True)
            nc.vector.tensor_copy(agI2[:, it, OUT:OUT + 1], f2p[:, :])
            transpose_to(agI2[:, it, 0:OUT], wh2b[:, isl], OUT, idb)
        agIn2 = dram.tile([NS, OUT + 1], BF16)
        nc.gpsimd.dma_start(
            agIn2[:, :].rearrange("(it p) c -> p it c", p=128), agI2[:, :, :])
        agOut2 = dram.tile([N, OUT + 1], BF16, addr_space="Shared")
        nc.gpsimd.collective_compute(
            "AllGather", ALU.bypass, replica_groups=RG,
            ins=[agIn2.opt()], outs=[agOut2.opt()])
        agO2 = gnn.tile([128, JT, OUT + 1], BF16)
        nc.gpsimd.dma_start(
            agO2[:, :, :], agOut2[:, :].rearrange("(jt p) c -> p jt c", p=128))
        f2of = gnn.tile([128, JT, 1], F32)
        nc.vector.tensor_copy(f2of[:, :, :], agO2[:, :, OUT:OUT + 1])

        un2 = pstile([64, NS], F32, "accS")
        z2p = pstile([1, NS], F32, "accS")
        for jb in range(2):
            jts = range(jb * 8, jb * 8 + 8)
            zs, ls, ems = {}, {}, {}
            for jt in jts:
                zs[jt] = sbG.tile([128, NS], F32, tag="zsb", bufs=9,
                                  name=f"oz_{jt}")
                nc.scalar.activation(zs[jt][:, :], f1ob[:, :], AF.Identity,
                                     bias=f2of[:, jt, :], scale=1.0)
            for jt in jts:
                ls[jt] = sbG.tile([128, NS], F32, tag="lsb", bufs=9,
                                  name=f"ol_{jt}")
                nc.vector.scalar_tensor_tensor(
                    ls[jt][:, :], zs[jt][:, :], 0.2, zs[jt][:, :],
                    op0=ALU.mult, op1=ALU.max)
            for jt in jts:
                zs[jt] = sbG.tile([128, NS], BF16, tag="esb", bufs=9,
                                  name=f"oe_{jt}")
                nc.scalar.activation(zs[jt][:, :], ls[jt][:, :], AF.Exp)
            for jt in jts:
                ems[jt] = sbG.tile([128, NS], BF16, tag="em", bufs=9,
                                   name=f"om_{jt}")
                nc.vector.tensor_mul(ems[jt][:, :], zs[jt][:, :],
                                     adjT[:, jt, :])
            for jt in jts:
                nc.tensor.matmul(un2[:, :], agO2[:, jt, 0:OUT], ems[jt][:, :],
                                 start=(jt == 0), stop=(jt == JT - 1))
                nc.tensor.matmul(z2p[:, :], ones_col[:, :], ems[jt][:, :],
                                 start=(jt == 0), stop=(jt == JT - 1))
        z2iv = sbG.tile([1, NS], F32, tag="ziv")
        nc.vector.reciprocal(z2iv[:, :], z2p[:, :])
        z2bc = pstile([64, NS], F32, "tp")
        nc.tensor.matmul(z2bc[:, :], ones_row_f[:, 0:64], z2iv[:, :],
                         start=True, stop=True)
        z2bs = sbG.tile([64, NS], F32, tag="z2bs")
        nc.vector.tensor_copy(z2bs[:, :], z2bc[:, :])
        ho2 = sbG.tile([64, NS], F32, tag="ho2")
        nc.vector.tensor_mul(ho2[:, :], un2[:, :], z2bs[:, :])
        mn2 = sbG.tile([64, NS], F32, tag="mn2")
        nc.vector.tensor_scalar_min(mn2[:, :], ho2[:, :], 0.0)
        ex2 = sbG.tile([64, NS], F32, tag="ex2")
        nc.scalar.activation(ex2[:, :], mn2[:, :], AF.Exp)
        ep2 = sbG.tile([64, NS], F32, tag="ep2")
        nc.vector.scalar_tensor_tensor(ep2[:, :], ho2[:, :], 0.0, ex2[:, :],
                                       op0=ALU.max, op1=ALU.add)
        gatx = gnn.tile([64, NS], F32)
        # lrelu(BN*(ep2-1), 0.01) = lrelu(BN*ep2 - BN, 0.01)
        nc.scalar.activation(gatx[:, :], ep2[:, :], AF.Lrelu,
                             bias=negbn[:, :], scale=BN, alpha=0.01)

        mark("fuse")
        # ---- fusion ----
        fu = sbG
        S = fu.tile([64, NS], F32, tag="fS")
        nc.vector.tensor_add(S[:, :], gatx[:, :], gcnx[:, :])
        base = fu.tile([64, NS], F32, tag="fb")
        nc.vector.tensor_scalar(base[:, :], S[:, :], fuse[:, 5:6],
                                fuse[:, 2:3], op0=ALU.mult, op1=ALU.add)
        g1 = fu.tile([64, NS], F32, tag="fg1")
        nc.vector.scalar_tensor_tensor(g1[:, :], gcnx[:, :], fuse[:, 0:1],
                                       base[:, :], op0=ALU.mult, op1=ALU.add)
        g2 = fu.tile([64, NS], F32, tag="fg2")
        nc.vector.scalar_tensor_tensor(g2[:, :], gatx[:, :], fuse[:, 1:2],
                                       g1[:, :], op0=ALU.mult, op1=ALU.add)
        mnf = fu.tile([64, NS], F32, tag="fmn")
        nc.vector.tensor_tensor(mnf[:, :], gcnx[:, :], gatx[:, :], op=ALU.min)
        mxf = fu.tile([64, NS], F32, tag="fmx")
        nc.vector.tensor_tensor(mxf[:, :], gcnx[:, :], gatx[:, :], op=ALU.max)
        g3 = fu.tile([64, NS], F32, tag="fg3")
        nc.vector.scalar_tensor_tensor(g3[:, :], mnf[:, :], fuse[:, 3:4],
                                       g2[:, :], op0=ALU.mult, op1=ALU.add)
        yf = fu.tile([64, NS], F32, tag="fyf")
        nc.vector.scalar_tensor_tensor(yf[:, :], mxf[:, :], fuse[:, 4:5],
                                       g3[:, :], op0=ALU.mult, op1=ALU.add)
        ybn = fu.tile([64, NS], F32, tag="fybn")
        nc.scalar.mul(ybn[:, :], yf[:, :], BN)
        yT = fu.tile([64, NS], F32, tag="fyT")
        nc.vector.scalar_tensor_tensor(yT[:, :], ybn[:, :], 0.2, ybn[:, :],
                                       op0=ALU.mult, op1=ALU.max)
        yTb = fu.tile([64, NS], BF16, tag="fyTb")
        nc.vector.tensor_copy(yTb[:, :], yT[:, :])
        # transpose to natural, AllGather y
        agYi = gnn.tile([128, 2, OUT], BF16)
        for it in range(2):
            transpose_to(agYi[:, it, :], yTb[:, it * 128:(it + 1) * 128],
                         OUT, idb)
        agYIn = dram.tile([NS, OUT], BF16)
        nc.gpsimd.dma_start(
            agYIn[:, :].rearrange("(it p) c -> p it c", p=128), agYi[:, :, :])
        yD = dram.tile([N, OUT], BF16, addr_space="Shared")
        nc.gpsimd.collective_compute(
            "AllGather", ALU.bypass, replica_groups=RG,
            ins=[agYIn.opt()], outs=[yD.opt()])
        nc.gpsimd.dma_start(
            ysb[:, :, :], yD[:, :].rearrange("(jt p) f -> p jt f", p=128))
        gnn_blk.close()

        mark("C")
        # ---- phase C: outT[f, p] = sum_n y[n, f] Q.T[n, p] ----
        with tc.tile_pool(name="psC", bufs=1, space="PSUM") as psC, \
             tc.tile_pool(name="sbO", bufs=1) as sbO:
            osb = sbO.tile([64, 16, 512], F32)
            for qr in range(4):
                psT = psC.tile([64, 4, 512], F32, tag="psT", bufs=2,
                               name=f"psT{qr}")
                for jt in range(JT):
                    for pb in range(4):
                        nc.tensor.matmul(
                            psT[:, pb, :], ysb[:, jt, :],
                            qtts[jt][qr][:, pb * 512:(pb + 1) * 512],
                            start=(jt == 0), stop=(jt == JT - 1))
                nc.vector.tensor_copy(osb[:, qr * 4:(qr + 1) * 4, :],
                                      psT[:, :, :])
            nc.sync.dma_start(
                out_d[:, :].rearrange("f (b c) -> f b c", c=512),
                osb[:, :, :])
        mark(None)

    nc.compile()
    return nc


def _prep_inputs(x, adj, Q, g1a_W, g1a_b, g1b_W, g1b_b, g2a_W, g2a_b,
                 g2b_W, g2b_b, gat_W, gat_a, out_W, out_a, Wf, bf,
                 conv_w, conv_b):
    bft = ml_dtypes.bfloat16
    f32 = np.float32
    xf = np.asarray(x, f32).reshape(HW, C)
    Qb = np.asarray(Q, f32).astype(bft)
    adj = np.asarray(adj, f32)

    w1a = (BN ** 3 * np.asarray(g1a_W, f32)).astype(bft)
    w1b = (BN * np.asarray(g1b_W, f32)).astype(bft)
    w2a = (BN * np.asarray(g2a_W, f32)).astype(bft)
    w2b = (BN * np.asarray(g2b_W, f32)).astype(bft)
    gatw = np.concatenate([BN * np.asarray(gat_W[k], f32)
                           for k in range(HEADS)], axis=1).astype(bft)
    gata1 = np.concatenate(
        [np.tile(np.asarray(gat_a[k, :NHID], f32)[:, None], (1, 128))
         for k in range(HEADS)], axis=1).astype(bft)
    gata2 = np.stack([np.asarray(gat_a[k, NHID:], f32)
                      for k in range(HEADS)], axis=1).astype(bft)
    outw = np.asarray(out_W, f32).astype(bft)
    outa1 = np.tile(np.asarray(out_a[:OUT], f32)[:, None], (1, 128)).astype(bft)
    outa2 = np.asarray(out_a[OUT:], f32)[:, None].astype(bft)
    b1a = np.asarray(g1a_b, f32)[:, None]
    b1b = (BN * np.asarray(g1b_b, f32))[:, None]
    b2a = np.asarray(g2a_b, f32)[:, None]
    b2b = (BN * np.asarray(g2b_b, f32))[:, None]
    cw = np.asarray(conv_w, f32)
    cb = float(np.asarray(conv_b, f32)[0])
    Wf = np.asarray(Wf, f32)
    bfv = np.asarray(bf, f32)
    fuse = np.stack([
        cw[2] * Wf[0], cw[2] * Wf[1],
        cw[2] * (bfv[0] + bfv[1]) + cb,
        np.full(OUT, cw[0], f32), np.full(OUT, cw[1], f32),
        np.full(OUT, cw.sum(), f32),
    ], axis=1).astype(f32)

    shared = dict(w1a=w1a, w1b=w1b, w2a=w2a, w2b=w2b, gatw=gatw,
                  gata1=gata1, gata2=gata2, outw=outw, outa1=outa1,
                  outa2=outa2, b1a=b1a, b1b=b1b, b2a=b2a, b2b=b2b, fuse=fuse)

    onespad = np.zeros((P, CP - C), f32)
    onespad[:, -1] = 1.0
    in_maps = []
    for c in range(NCORES):
        m = dict(shared)
        psl = slice(c * P, (c + 1) * P)
        m["xq"] = np.ascontiguousarray(
            np.concatenate([xf[psl], onespad], axis=1)).astype(bft)
        m["q"] = np.ascontiguousarray(Qb[psl])
        m["qt"] = np.ascontiguousarray(Qb[psl].T)
        m["adjT"] = np.ascontiguousarray(
            adj[c * NS:(c + 1) * NS, :].T.astype(bft))
        in_maps.append(m)
    return in_maps


def _get_nc():
    if "nc" not in _CACHE:
        _CACHE["nc"] = _build()
    return _CACHE["nc"]


def run_traced(trace=False, **inputs):
    nc = _get_nc()
    in_maps = _prep_inputs(**inputs)
    res = run_bass_kernel_spmd(nc, in_maps, core_ids=list(range(NCORES)),
                               trace=trace)
    out = np.concatenate([res.results[c]["out"].T for c in range(NCORES)],
                         axis=0)
    return out, res


def kernel(**inputs):
    out, _ = run_traced(trace=False, **inputs)
    return out

